# revision 1
# baseline (speedup 1.0000x reference)
"""LIF-with-sparse-spikes kernel for Trainium2, 8 NeuronCores.

Strategy (batch data-parallel, per sharding hint):
 - 8 cores x 4 batch items each; the full 4096-unit state lives on each core
   for its batch shard, so no cross-core communication is needed.
 - The synaptic gather  syn[t,b,:] = sum_{s<n} Wt[ids[t,b,s]]  is reformulated
   as a dense matmul  syn = W'^T-tiles @ X  where X[k, tb] is the per-(t,b)
   histogram (count) of spike ids (duplicates counted, invalid slots dropped)
   and W' = (1-decay)[:,None] * W.  This replaces ~8.3 GB of row gathers with
   a 275 GFLOP fp32 matmul streaming W once per core.
 - The time recurrence state = decay*state + syn' is computed with the DVE
   tensor_tensor_scan instruction (exact fp32 sequential semantics) along the
   free (time) axis — valid as long as no neuron crosses threshold (verified
   on-device via spike counts; see fallback below).
 - Spike counts (num_out_spikes) are computed on device:
   acc += (state > threshold), then a ones-vector matmul reduces partitions.
 - For this input distribution state sigma ~0.065 << threshold 1.0, so no
   spikes ever occur and out_spike_ids === 0, num_out === 0.  If the device
   counts ever report a spike, kernel() falls back to an exact numpy
   re-computation (reset semantics included) so correctness never depends on
   that data property.
"""

import sys

sys.path.insert(0, "/opt/trn_rl_repo")

import numpy as np

T, B, UNITS, IN_DIM = 256, 32, 4096, 4096
S_IN = 128
NCORES = 8
BLOC = B // NCORES          # 4 batch items per core
TB = BLOC * T               # 1024 (b, t) pairs per core, b-major
MT = UNITS // 128           # 32 output tiles
KT = IN_DIM // 128          # 32 contraction tiles

_compiled = None


def _build_program():
    import concourse.bacc as bacc
    import concourse.tile as tile
    import concourse.mybir as mybir
    from concourse.mybir import AluOpType as Alu

    f32 = mybir.dt.float32
    nc = bacc.Bacc("TRN2", target_bir_lowering=False, debug=False,
                   enable_asserts=True, num_devices=NCORES)

    w_t = nc.dram_tensor("w_tiles", [MT, KT, 128, 128], f32, kind="ExternalInput").ap()
    x_t = nc.dram_tensor("x_tiles", [KT, 128, TB], f32, kind="ExternalInput").ap()
    decay_t = nc.dram_tensor("decay_t", [128, MT], f32, kind="ExternalInput").ap()
    th_t = nc.dram_tensor("th_t", [128, MT], f32, kind="ExternalInput").ap()
    init_t = nc.dram_tensor("init_t", [128, MT * BLOC], f32, kind="ExternalInput").ap()
    out_states = nc.dram_tensor("out_states", [BLOC, UNITS, T], f32, kind="ExternalOutput").ap()
    out_counts = nc.dram_tensor("out_counts", [1, TB], f32, kind="ExternalOutput").ap()

    with tile.TileContext(nc) as tc:
        with (
            tc.tile_pool(name="xpool", bufs=1) as xpool,
            tc.tile_pool(name="wpool", bufs=2) as wpool,
            tc.tile_pool(name="spool", bufs=2) as spool,
            tc.tile_pool(name="misc", bufs=1) as misc,
            tc.tile_pool(name="psum", bufs=2, space="PSUM") as ppool,
            tc.tile_pool(name="psum1", bufs=1, space="PSUM") as ppool1,
        ):
            x_sb = xpool.tile([128, KT, TB], f32)
            nc.sync.dma_start(out=x_sb[:], in_=x_t.rearrange("k p t -> p k t"))

            decay_sb = misc.tile([128, MT], f32)
            nc.sync.dma_start(out=decay_sb[:], in_=decay_t[:])
            th_sb = misc.tile([128, MT], f32)
            nc.sync.dma_start(out=th_sb[:], in_=th_t[:])
            init_sb = misc.tile([128, MT * BLOC], f32)
            nc.sync.dma_start(out=init_sb[:], in_=init_t[:])

            acc = misc.tile([128, TB], f32)
            nc.vector.memset(acc[:], 0.0)
            ones = misc.tile([128, 1], f32)
            nc.vector.memset(ones[:], 1.0)

            for mt in range(MT):
                w_sb = wpool.tile([128, KT, 128], f32, tag="w")
                nc.sync.dma_start(out=w_sb[:], in_=w_t[mt].rearrange("k p u -> p k u"))

                psum = ppool.tile([128, TB], f32, tag="syn")
                for kt in range(KT):
                    for h in range(TB // 512):
                        nc.tensor.matmul(
                            out=psum[:, h * 512:(h + 1) * 512],
                            lhsT=w_sb[:, kt, :],
                            rhs=x_sb[:, kt, h * 512:(h + 1) * 512],
                            start=(kt == 0), stop=(kt == KT - 1),
                        )

                st_sb = spool.tile([128, TB], f32, tag="st")
                for b in range(BLOC):
                    nc.vector.tensor_tensor_scan(
                        out=st_sb[:, b * T:(b + 1) * T],
                        data0=decay_sb[:, mt:mt + 1].to_broadcast([128, T]),
                        data1=psum[:, b * T:(b + 1) * T],
                        initial=init_sb[:, mt * BLOC + b:mt * BLOC + b + 1],
                        op0=Alu.mult, op1=Alu.add,
                    )

                spk = spool.tile([128, TB], f32, tag="spk")
                nc.vector.tensor_scalar(
                    out=spk[:], in0=st_sb[:], scalar1=th_sb[:, mt:mt + 1],
                    scalar2=None, op0=Alu.is_gt,
                )
                nc.vector.tensor_tensor(out=acc[:], in0=acc[:], in1=spk[:], op=Alu.add)

                for b in range(BLOC):
                    nc.sync.dma_start(
                        out=out_states[b, mt * 128:(mt + 1) * 128, :],
                        in_=st_sb[:, b * T:(b + 1) * T],
                    )

            cnt_sb = misc.tile([1, TB], f32)
            for h in range(TB // 512):
                cpsum = ppool1.tile([1, 512], f32, tag=f"cnt{h}")
                nc.tensor.matmul(out=cpsum[:], lhsT=ones[:],
                                 rhs=acc[:, h * 512:(h + 1) * 512],
                                 start=True, stop=True)
                nc.vector.tensor_scalar(
                    out=cnt_sb[:, h * 512:(h + 1) * 512], in0=cpsum[:],
                    scalar1=128.0, scalar2=None, op0=Alu.min,
                )
            nc.sync.dma_start(out=out_counts[:], in_=cnt_sb[:])

    nc.compile()
    return nc


def _host_prep(weights, init_state, inp_spike_ids, num_inp_spikes, decay_constants,
               thresholds):
    w = np.asarray(weights, np.float32)
    dec = np.asarray(decay_constants, np.float32)
    th = np.asarray(thresholds, np.float32)
    ids = np.asarray(inp_spike_ids)
    n = np.asarray(num_inp_spikes)
    init = np.asarray(init_state, np.float32)

    wp = (1.0 - dec)[:, None] * w                       # [u, k]
    wpt = np.ascontiguousarray(wp.T)                    # [k, u]
    w_tiles = np.ascontiguousarray(
        wpt.reshape(KT, 128, MT, 128).transpose(2, 0, 1, 3))  # [mt, kt, kin, u]

    decay_t = np.ascontiguousarray(dec.reshape(MT, 128).T)   # [128, mt]
    th_t = np.ascontiguousarray(th.reshape(MT, 128).T)

    slot = np.arange(S_IN)
    in_maps = []
    for c in range(NCORES):
        bsl = slice(c * BLOC, (c + 1) * BLOC)
        ids_c = ids[:, bsl, :]                           # [T, BLOC, S]
        n_c = n[:, bsl, 0]                               # [T, BLOC]
        valid = slot[None, None, :] < n_c[:, :, None]
        tb = (np.arange(BLOC)[None, :, None] * T + np.arange(T)[:, None, None])
        flat = (tb * IN_DIM + ids_c)[valid]
        X = np.bincount(flat.ravel(), minlength=TB * IN_DIM).reshape(TB, IN_DIM)
        x_tiles = np.ascontiguousarray(X.T.astype(np.float32).reshape(KT, 128, TB))

        init_c = np.ascontiguousarray(
            init[bsl].reshape(BLOC, MT, 128).transpose(2, 1, 0).reshape(128, MT * BLOC))
        in_maps.append(dict(w_tiles=w_tiles, x_tiles=x_tiles, decay_t=decay_t,
                            th_t=th_t, init_t=init_c))
    return in_maps


def _exact_fallback(weights, init_state, inp_spike_ids, num_inp_spikes,
                    decay_constants, thresholds, size_sparse_out):
    w = np.asarray(weights, np.float64)
    dec = np.asarray(decay_constants, np.float64)
    th = np.asarray(thresholds, np.float64)
    ids = np.asarray(inp_spike_ids)
    n = np.asarray(num_inp_spikes)
    s_out = int(size_sparse_out)
    wt = w.T
    state = np.asarray(init_state, np.float64).copy()
    oi = np.zeros((T, B, s_out), np.float32)
    no = np.zeros((T, B, 1), np.float32)
    st = np.zeros((T, B, UNITS), np.float32)
    for t in range(T):
        syn = np.zeros((B, UNITS))
        for b in range(B):
            k = int(n[t, b, 0])
            if k:
                syn[b] = wt[ids[t, b, :k]].sum(0)
        state = dec * state + (1.0 - dec) * syn
        spiked = state > th
        st[t] = state.astype(np.float32)
        for b in range(B):
            idx = np.nonzero(spiked[b])[0][:s_out]
            oi[t, b, :len(idx)] = idx.astype(np.float32)
            no[t, b, 0] = min(int(spiked[b].sum()), s_out)
        state = np.where(spiked, 0.0, state)
    return oi, no, st


def kernel(weights, init_state, inp_spike_ids, num_inp_spikes, decay_constants,
           thresholds, size_sparse_out):
    global _compiled
    from concourse.bass_utils import run_bass_kernel_spmd

    if _compiled is None:
        _compiled = _build_program()
    nc = _compiled

    in_maps = _host_prep(weights, init_state, inp_spike_ids, num_inp_spikes,
                         decay_constants, thresholds)
    res = run_bass_kernel_spmd(nc, in_maps, core_ids=list(range(NCORES)))

    s_out = int(size_sparse_out)
    states = np.zeros((T, B, UNITS), np.float32)
    num_out = np.zeros((T, B, 1), np.float32)
    for c in range(NCORES):
        r = res.results[c]
        states[:, c * BLOC:(c + 1) * BLOC, :] = r["out_states"].transpose(2, 0, 1)
        num_out[:, c * BLOC:(c + 1) * BLOC, 0] = r["out_counts"].reshape(BLOC, T).T
    out_ids = np.zeros((T, B, s_out), np.float32)

    if num_out.any():
        # a neuron crossed threshold: the no-reset scan shortcut is invalid
        # for this input — recompute everything exactly on host.
        return _exact_fallback(weights, init_state, inp_spike_ids,
                               num_inp_spikes, decay_constants, thresholds,
                               size_sparse_out)
    return out_ids, num_out, states


# revision 2
# speedup vs baseline: 2.0111x; 2.0111x over previous
"""LIF-with-sparse-spikes kernel for Trainium2, 8 NeuronCores.

Strategy (batch data-parallel, per sharding hint):
 - 8 cores x 4 batch items each; the full 4096-unit state lives on each core
   for its batch shard, so no cross-core communication is needed.
 - The synaptic gather  syn[t,b,:] = sum_{s<n} Wt[ids[t,b,s]]  is reformulated
   as a dense matmul  syn = W'^T-tiles @ X  where X[k, tb] is the per-(t,b)
   histogram (count) of spike ids (duplicates counted, invalid slots dropped)
   and W' = (1-decay)[:,None] * W.  This replaces ~8.3 GB of row gathers with
   a 275 GFLOP fp32 matmul streaming W once per core.
 - The time recurrence state = decay*state + syn' is computed with the DVE
   tensor_tensor_scan instruction (exact fp32 sequential semantics) along the
   free (time) axis — valid as long as no neuron crosses threshold (verified
   on-device via spike counts; see fallback below).
 - Spike counts (num_out_spikes) are computed on device:
   acc += (state > threshold), then a ones-vector matmul reduces partitions.
 - For this input distribution state sigma ~0.065 << threshold 1.0, so no
   spikes ever occur and out_spike_ids === 0, num_out === 0.  If the device
   counts ever report a spike, kernel() falls back to an exact numpy
   re-computation (reset semantics included) so correctness never depends on
   that data property.
"""

import sys

sys.path.insert(0, "/opt/trn_rl_repo")

import numpy as np

T, B, UNITS, IN_DIM = 256, 32, 4096, 4096
S_IN = 128
NCORES = 8
BLOC = B // NCORES          # 4 batch items per core
TB = BLOC * T               # 1024 (b, t) pairs per core, b-major
MT = UNITS // 128           # 32 output tiles
KT = IN_DIM // 128          # 32 contraction tiles

_compiled = None


def _build_program():
    import concourse.bacc as bacc
    import concourse.tile as tile
    import concourse.mybir as mybir
    from concourse.mybir import AluOpType as Alu

    f32 = mybir.dt.float32
    bf16 = mybir.dt.bfloat16
    nc = bacc.Bacc("TRN2", target_bir_lowering=False, debug=False,
                   enable_asserts=True, num_devices=NCORES)

    # W' split into hi + lo bf16 planes (stacked along a doubled K-tile dim):
    # tiles [mt, 0:KT] = bf16(W'), tiles [mt, KT:2KT] = bf16(W' - hi).  Both
    # planes accumulate into the same fp32 PSUM, recovering ~2^-18 relative
    # weight precision while streaming the PE at the bf16 rate (2 cols/cyc)
    # for the same total bytes as a single fp32 pass.
    w_t = nc.dram_tensor("w_tiles", [MT, 2 * KT, 128, 128], bf16, kind="ExternalInput").ap()
    x_t = nc.dram_tensor("x_tiles", [KT, 128, TB], bf16, kind="ExternalInput").ap()
    decay_t = nc.dram_tensor("decay_t", [128, MT], f32, kind="ExternalInput").ap()
    th_t = nc.dram_tensor("th_t", [128, MT], f32, kind="ExternalInput").ap()
    init_t = nc.dram_tensor("init_t", [128, MT * BLOC], f32, kind="ExternalInput").ap()
    out_states = nc.dram_tensor("out_states", [BLOC, UNITS, T], f32, kind="ExternalOutput").ap()
    out_counts = nc.dram_tensor("out_counts", [1, TB], f32, kind="ExternalOutput").ap()

    with tile.TileContext(nc) as tc:
        with (
            tc.tile_pool(name="xpool", bufs=1) as xpool,
            tc.tile_pool(name="wpool", bufs=2) as wpool,
            tc.tile_pool(name="spool", bufs=2) as spool,
            tc.tile_pool(name="misc", bufs=1) as misc,
            tc.tile_pool(name="psum", bufs=2, space="PSUM") as ppool,
            tc.tile_pool(name="psum1", bufs=1, space="PSUM") as ppool1,
        ):
            x_sb = xpool.tile([128, KT, TB], bf16)
            nc.sync.dma_start(out=x_sb[:], in_=x_t.rearrange("k p t -> p k t"))

            decay_sb = misc.tile([128, MT], f32)
            nc.sync.dma_start(out=decay_sb[:], in_=decay_t[:])
            th_sb = misc.tile([128, MT], f32)
            nc.sync.dma_start(out=th_sb[:], in_=th_t[:])
            init_sb = misc.tile([128, MT * BLOC], f32)
            nc.sync.dma_start(out=init_sb[:], in_=init_t[:])

            acc = misc.tile([128, TB], f32)
            nc.vector.memset(acc[:], 0.0)
            ones = misc.tile([128, 1], f32)
            nc.vector.memset(ones[:], 1.0)

            for mt in range(MT):
                w_sb = wpool.tile([128, 2 * KT, 128], bf16, tag="w")
                nc.sync.dma_start(out=w_sb[:], in_=w_t[mt].rearrange("k p u -> p k u"))

                psum = ppool.tile([128, TB], f32, tag="syn")
                for kt2 in range(2 * KT):
                    for h in range(TB // 512):
                        nc.tensor.matmul(
                            out=psum[:, h * 512:(h + 1) * 512],
                            lhsT=w_sb[:, kt2, :],
                            rhs=x_sb[:, kt2 % KT, h * 512:(h + 1) * 512],
                            start=(kt2 == 0), stop=(kt2 == 2 * KT - 1),
                        )

                st_sb = spool.tile([128, TB], f32, tag="st")
                for b in range(BLOC):
                    nc.vector.tensor_tensor_scan(
                        out=st_sb[:, b * T:(b + 1) * T],
                        data0=decay_sb[:, mt:mt + 1].to_broadcast([128, T]),
                        data1=psum[:, b * T:(b + 1) * T],
                        initial=init_sb[:, mt * BLOC + b:mt * BLOC + b + 1],
                        op0=Alu.mult, op1=Alu.add,
                    )

                spk = spool.tile([128, TB], f32, tag="spk")
                nc.vector.tensor_scalar(
                    out=spk[:], in0=st_sb[:], scalar1=th_sb[:, mt:mt + 1],
                    scalar2=None, op0=Alu.is_gt,
                )
                nc.vector.tensor_tensor(out=acc[:], in0=acc[:], in1=spk[:], op=Alu.add)

                for b in range(BLOC):
                    nc.sync.dma_start(
                        out=out_states[b, mt * 128:(mt + 1) * 128, :],
                        in_=st_sb[:, b * T:(b + 1) * T],
                    )

            cnt_sb = misc.tile([1, TB], f32)
            for h in range(TB // 512):
                cpsum = ppool1.tile([1, 512], f32, tag=f"cnt{h}")
                nc.tensor.matmul(out=cpsum[:], lhsT=ones[:],
                                 rhs=acc[:, h * 512:(h + 1) * 512],
                                 start=True, stop=True)
                nc.vector.tensor_scalar(
                    out=cnt_sb[:, h * 512:(h + 1) * 512], in0=cpsum[:],
                    scalar1=128.0, scalar2=None, op0=Alu.min,
                )
            nc.sync.dma_start(out=out_counts[:], in_=cnt_sb[:])

    nc.compile()
    return nc


def _host_prep(weights, init_state, inp_spike_ids, num_inp_spikes, decay_constants,
               thresholds):
    w = np.asarray(weights, np.float32)
    dec = np.asarray(decay_constants, np.float32)
    th = np.asarray(thresholds, np.float32)
    ids = np.asarray(inp_spike_ids)
    n = np.asarray(num_inp_spikes)
    init = np.asarray(init_state, np.float32)

    import ml_dtypes
    bf = ml_dtypes.bfloat16
    wp = (1.0 - dec)[:, None] * w                       # [u, k]
    wpt = np.ascontiguousarray(wp.T)                    # [k, u]
    wt4 = np.ascontiguousarray(
        wpt.reshape(KT, 128, MT, 128).transpose(2, 0, 1, 3))  # [mt, kt, kin, u]
    w_hi = wt4.astype(bf)
    w_lo = (wt4 - w_hi.astype(np.float32)).astype(bf)
    w_tiles = np.ascontiguousarray(
        np.concatenate([w_hi, w_lo], axis=1))           # [mt, 2*kt, kin, u]

    decay_t = np.ascontiguousarray(dec.reshape(MT, 128).T)   # [128, mt]
    th_t = np.ascontiguousarray(th.reshape(MT, 128).T)

    slot = np.arange(S_IN)
    in_maps = []
    for c in range(NCORES):
        bsl = slice(c * BLOC, (c + 1) * BLOC)
        ids_c = ids[:, bsl, :]                           # [T, BLOC, S]
        n_c = n[:, bsl, 0]                               # [T, BLOC]
        valid = slot[None, None, :] < n_c[:, :, None]
        tb = (np.arange(BLOC)[None, :, None] * T + np.arange(T)[:, None, None])
        flat = (tb * IN_DIM + ids_c)[valid]
        X = np.bincount(flat.ravel(), minlength=TB * IN_DIM).reshape(TB, IN_DIM)
        x_tiles = np.ascontiguousarray(X.T.astype(bf).reshape(KT, 128, TB))

        init_c = np.ascontiguousarray(
            init[bsl].reshape(BLOC, MT, 128).transpose(2, 1, 0).reshape(128, MT * BLOC))
        in_maps.append(dict(w_tiles=w_tiles, x_tiles=x_tiles, decay_t=decay_t,
                            th_t=th_t, init_t=init_c))
    return in_maps


def _exact_fallback(weights, init_state, inp_spike_ids, num_inp_spikes,
                    decay_constants, thresholds, size_sparse_out):
    w = np.asarray(weights, np.float64)
    dec = np.asarray(decay_constants, np.float64)
    th = np.asarray(thresholds, np.float64)
    ids = np.asarray(inp_spike_ids)
    n = np.asarray(num_inp_spikes)
    s_out = int(size_sparse_out)
    wt = w.T
    state = np.asarray(init_state, np.float64).copy()
    oi = np.zeros((T, B, s_out), np.float32)
    no = np.zeros((T, B, 1), np.float32)
    st = np.zeros((T, B, UNITS), np.float32)
    for t in range(T):
        syn = np.zeros((B, UNITS))
        for b in range(B):
            k = int(n[t, b, 0])
            if k:
                syn[b] = wt[ids[t, b, :k]].sum(0)
        state = dec * state + (1.0 - dec) * syn
        spiked = state > th
        st[t] = state.astype(np.float32)
        for b in range(B):
            idx = np.nonzero(spiked[b])[0][:s_out]
            oi[t, b, :len(idx)] = idx.astype(np.float32)
            no[t, b, 0] = min(int(spiked[b].sum()), s_out)
        state = np.where(spiked, 0.0, state)
    return oi, no, st


def kernel(weights, init_state, inp_spike_ids, num_inp_spikes, decay_constants,
           thresholds, size_sparse_out):
    global _compiled
    from concourse.bass_utils import run_bass_kernel_spmd

    if _compiled is None:
        _compiled = _build_program()
    nc = _compiled

    in_maps = _host_prep(weights, init_state, inp_spike_ids, num_inp_spikes,
                         decay_constants, thresholds)
    res = run_bass_kernel_spmd(nc, in_maps, core_ids=list(range(NCORES)))

    s_out = int(size_sparse_out)
    states = np.zeros((T, B, UNITS), np.float32)
    num_out = np.zeros((T, B, 1), np.float32)
    for c in range(NCORES):
        r = res.results[c]
        states[:, c * BLOC:(c + 1) * BLOC, :] = r["out_states"].transpose(2, 0, 1)
        num_out[:, c * BLOC:(c + 1) * BLOC, 0] = r["out_counts"].reshape(BLOC, T).T
    out_ids = np.zeros((T, B, s_out), np.float32)

    if num_out.any():
        # a neuron crossed threshold: the no-reset scan shortcut is invalid
        # for this input — recompute everything exactly on host.
        return _exact_fallback(weights, init_state, inp_spike_ids,
                               num_inp_spikes, decay_constants, thresholds,
                               size_sparse_out)
    return out_ids, num_out, states
